# revision 34
# baseline (speedup 1.0000x reference)
"""Expert-parallel MoE layer for Trainium2 (8 NeuronCores, one expert per core).

Host side (numpy): router logits, exact top-2 dispatch, p0 weights, and the
scatter-add combine. Device side (Bass/Tile, SPMD over 8 cores): the dense FFN
y = gelu(x @ W1[e] + b1[e]) @ W2[e] over the tokens routed to expert e,
computed with fp16 operands (fp32 PSUM accumulation).

v4 layout: tokens ride the MOVING dim of BOTH GEMMs (per-core token capacity =
max expert load, GEMM2 consumes h^T directly, y^T written once). Weights are
host-prepacked into their exact SBUF images -- w1 fs-major [P, KF, KH*P], w2
n-major [P, NH, KF*P] -- so every weight DMA is a contiguous row-slice with
2KB+ runs per partition (v2's [H,F]-layout slices moved 256-512B packets and
the startup fs0 chunk alone took ~6us of packet-rate-bound DMA).

Per token group (TT=512, tail-sized last group):
  GEMM1: psum[f128, t] = sum_k w1[k, f128]^T x^T[k, t]   (8 k-chunks over H)
  gelu+bias -> h[f128-chunk, t] fp16                     (32 f-chunks)
  GEMM2: psum[h'128, t] = sum_k2 w2[k2, h'128]^T h[k2, t] (32 k2-chunks over F)
  copy -> y^T stage -> single DMA store per group

Overflow (experts loaded past cap): excess tokens' FFN is F-sliced across all
8 cores (each core does F/8 columns for ALL overflow tokens; partial y summed
on host). v4 runs overflow GEMM2 strip-major (for n: for seg) into one
[P, NH, NO] stage with one sync-issued store per n-strip -- v2's 8-stores-per-
seg on sync saturated its sequencer (~1.2us per dma_start) and backpressured
the PE for ~6us.
"""

import numpy as np

B, S, H, E, F = 4, 2048, 1024, 8, 4096
T = B * S
P = 128
TT = 512            # token group size (moving free dim of both GEMMs)
KH = H // P         # 8  k-chunks over H  (GEMM1 contraction)
KF = F // P         # 32 k-chunks over F  (GEMM2 contraction)
NH = H // P         # 8  output h'-chunks of GEMM2

_cache = {}


def _spill_waits(nc, mybir, max_waits=1):
    """walrus CoreV2/V3 codegen rejects instructions with >1 semaphore wait
    ("Too many sync wait commands"). Move excess waits onto same-engine no-ops
    inserted right before the instruction (sequencers run in order, so this is
    equivalent)."""
    for fn in nc.m.functions:
        for blk in fn.blocks:
            out = []
            changed = False
            for inst in blk.instructions:
                si = getattr(inst, "sync_info", None)
                if si is not None and len(si.on_wait) > max_waits:
                    spill = si.on_wait[: len(si.on_wait) - max_waits]
                    keep = si.on_wait[len(si.on_wait) - max_waits:]
                    for w in spill:
                        nop = mybir.InstNoOp(
                            name=nc.get_next_instruction_name(),
                            engine=inst.engine,
                            ins=[],
                            outs=[],
                        )
                        nop.sync_info = mybir.SyncInfo(on_wait=[w], on_update=[])
                        out.append(nop)
                    inst.sync_info = mybir.SyncInfo(on_wait=keep, on_update=si.on_update)
                    changed = True
                out.append(inst)
            if changed:
                blk.instructions = out


def _build(cap, segs=()):
    """cap = main-segment capacity (own-expert tokens per core). segs = widths
    of the overflow segments (<=TT each), widest first. SPMD-uniform: the
    per-core difference lives entirely in the input DATA."""
    import concourse.bass as bass
    import concourse.mybir as mybir
    from concourse import tile

    F32 = mybir.dt.float32
    SDT = mybir.dt.float16
    GELU = mybir.ActivationFunctionType.Gelu_apprx_tanh

    S = len(segs)
    NO = sum(segs)
    SW = F // E  # per-core F-slice width (512)
    assert 4 * S <= KF and S * SW <= F
    capx = cap + NO

    nc = bass.Bass()
    xt = nc.declare_dram_parameter("xt", [H, capx], SDT, isOutput=False)
    # host-prepacked SBUF images (see _prepare): flat [P, ...] row layouts
    w1 = nc.declare_dram_parameter("w1", [P, KF * KH * P], SDT, isOutput=False)
    w2 = nc.declare_dram_parameter("w2", [P, NH * KF * P], SDT, isOutput=False)
    b1s = nc.declare_dram_parameter("b1s", [P, KF + 4 * S], F32, isOutput=False)
    yt = nc.declare_dram_parameter("yt", [H, capx], SDT, isOutput=True)
    if S:
        w1o = nc.declare_dram_parameter(
            "w1o", [P, S * 4 * KH * P], SDT, isOutput=False
        )
        w2o = nc.declare_dram_parameter("w2o", [S * SW, H], SDT, isOutput=False)

    # token groups: the odd-sized group goes FIRST when it's big enough to
    # pace the w1 stream (256KB per fs chunk each 8*(w+6) cycles needs
    # w >= ~320 at ~250GB/s) -- a smaller group 0 shrinks the startup-critical
    # x burst; otherwise full groups first, tail last
    n_g = -(-cap // TT)
    w0 = cap - TT * (n_g - 1)
    if n_g > 1 and 384 <= w0 < TT:
        widths = [w0] + [TT] * (n_g - 1)
    else:
        widths = [TT] * (n_g - 1) + [w0]
    groups = []
    o = 0
    for tt in widths:
        groups.append((o, tt))
        o += tt
    g0w = groups[0][1]

    xsrc = xt.rearrange("(c p) t -> p c t", p=P)
    w1src = w1.rearrange("p (a r) -> p a r", a=KF)       # [P, KF, KH*P]
    w2src = w2.rearrange("p (a r) -> p a r", a=NH)       # [P, NH, KF*P]
    ydst = yt.rearrange("(c p) t -> p c t", p=P)

    with tile.TileContext(nc) as tc:
        with (
            tc.tile_pool(name="wts", bufs=1) as wts,
            tc.tile_pool(name="ps", bufs=4, space="PSUM") as ps,
        ):
            ps1 = ps2 = ps
            # Startup DMA orchestration, rules learned from traces:
            # (1) each dma_start costs ~0.8-1.2us of the ISSUING engine's
            # sequencer -- keep the startup-critical path to few, large DMAs
            # and put them first in their queues; (2) emission order =
            # completion order per ring; (3) the 2MB x group-0 burst is
            # HBM-rate-bound on one 16-engine ring, so split it across the
            # scalar AND sync rings; (4) w1 is consumed one fs chunk per
            # ~1.73us from PE start, so fs0 rides alone (small, early) and
            # the rest follows in 2MB slabs in consumption order.
            w1r = wts.tile([P, KF, KH * P], SDT, tag="w1r")
            x_all = wts.tile([P, KH, cap], SDT, tag="x")
            b1t = wts.tile([P, KF + 4 * S], F32, tag="b1")
            w2r = wts.tile([P, NH, KF * P], SDT, tag="w2r")
            warm = wts.tile([P, P + TT], SDT, tag="warm")
            if S:
                xo_t = wts.tile([P, KH, NO], SDT, tag="xo")
            nc.gpsimd.memset(warm[:], 0)
            nc.scalar.dma_start(x_all[:, :4, :g0w], xsrc[:, :4, :g0w])
            # b1 (16KB) rides gpsimd's slow 1-engine ring: off the critical
            # sync queue, still lands long before the first gelu
            nc.gpsimd.dma_start(b1t[:], b1s[:])
            nc.sync.dma_start(w1r[:, 0:1, :], w1src[:, 0:1, :])
            nc.sync.dma_start(x_all[:, 4:, :g0w], xsrc[:, 4:, :g0w])
            for a, b in [(1, 3), (3, 6), (6, 10), (10, 16), (16, 24), (24, 32)]:
                nc.sync.dma_start(w1r[:, a:b, :], w1src[:, a:b, :])
            # w2 n-chunk n is first read at GEMM2(g0) + n*6.9us: easy deadlines
            for a, b in [(0, 2), (2, 4), (4, 6), (6, 8)]:
                nc.sync.dma_start(w2r[:, a:b, :], w2src[:, a:b, :])
            # x beyond group 0 is needed only from g1 (~120us): dead last
            rest = (cap - g0w + 1) // 2
            nc.sync.dma_start(
                x_all[:, :, g0w:g0w + rest], xsrc[:, :, g0w:g0w + rest]
            )
            nc.sync.dma_start(
                x_all[:, :, g0w + rest:cap], xsrc[:, :, g0w + rest:cap]
            )
            if S:
                nc.sync.dma_start(xo_t[:], xsrc[:, :, cap:capx])

            # PE clock warm-up: the HAM clock gate holds the PE at 1.2 GHz
            # until it has seen ~3.4us of sustained matmul activity, and the
            # first real matmul can't start until w1/x land (~14us). Run
            # dependency-free dummy matmuls (one stationary load, moving 512)
            # through that window so the real GEMM starts at 2.4 GHz.
            for _ in range(17):
                pw = ps2.tile([P, TT], F32, tag="pt2")
                nc.tensor.matmul(
                    pw[:], warm[:, 0:P], warm[:, P:P + TT], start=True, stop=True
                )

            # overflow-weight homes (created lazily at the last group's GEMM2,
            # after every x / w1r read is emitted, so the rotation WAR is
            # exactly "main loop done with that region")
            xch = min((KH * cap) // H, 4 * S) if S else 0
            assert 4 * S - xch <= KF - 4 * S
            if S:
                w1o_src = w1o.rearrange("p (a r) -> p a r", a=S * 4)
                w2o_src = w2o.rearrange("(c p) h -> p c h", p=P)
            ovt = {}

            h = wts.tile([P, KF, TT], SDT, tag="h")
            for gi, (t0, tt) in enumerate(groups):
                # GEMM1: h^T[f, t] = gelu(sum_k W1[k, f] * x^T[k, t] + b1[f])
                for fs in range(KF):
                    pt = ps1.tile([P, TT], F32, tag="pt1")
                    for k in range(KH):
                        nc.tensor.matmul(
                            pt[:, :tt],
                            w1r[:, fs, k * P:(k + 1) * P],
                            x_all[:, k, t0:t0 + tt],
                            start=(k == 0),
                            stop=(k == KH - 1),
                        )
                    nc.scalar.activation(
                        h[:, fs, :tt], pt[:, :tt], GELU, bias=b1t[:, fs:fs + 1]
                    )
                # GEMM2: y^T[h', t] = sum_k2 W2[k2, h'] * h^T[k2, t]
                # per-h'-chunk staging+store so the tail drains during the
                # last copies (and the stage stays at 4x1KB of SBUF)
                for n in range(NH):
                    pt2 = ps2.tile([P, TT], F32, tag="pt2")
                    for k2 in range(KF):
                        nc.tensor.matmul(
                            pt2[:, :tt],
                            w2r[:, n, k2 * P:(k2 + 1) * P],
                            h[:, k2, :tt],
                            start=(k2 == 0),
                            stop=(k2 == KF - 1),
                        )
                    stage = wts.tile([P, TT], SDT, tag="stage", bufs=2)
                    nc.vector.tensor_copy(stage[:, :tt], pt2[:, :tt])
                    nc.sync.dma_start(
                        ydst[:, n, t0:t0 + tt], stage[:, :tt]
                    )
                    # w2o prefetch, emitted here so sync's sequencer reaches
                    # it right when the x / w1r WARs clear (~end of this
                    # group's GEMM1): 10MB of overflow weights on the scalar
                    # ring alone finished ~15us into overflow GEMM2 and
                    # stalled its late strips
                    if S and gi == len(groups) - 1 and n == 0:
                        w1o_t = wts.tile([P, KF, KH * P], SDT, tag="w1r")
                        ovt["w1o"] = w1o_t
                        nc.scalar.dma_start(
                            w1o_t[:, : S * 4, :], w1o_src[:]
                        )
                        if xch:
                            w2o_t = wts.tile([P, xch, H], SDT, tag="x")
                            ovt["w2o"] = w2o_t
                            nc.sync.dma_start(
                                w2o_t[:], w2o_src[:, :xch, :]
                            )
                    if S and gi == len(groups) - 1 and n == 1 and 4 * S > xch:
                        nc.sync.dma_start(
                            ovt["w1o"][:, 4 * S:8 * S - xch, :],
                            w2o_src[:, xch:4 * S, :],
                        )

            if S:
                # Overflow phase: this core's F-slice of the FFN for the
                # overflow tokens of ALL overloaded experts. The slice weights
                # are DMA'd (on the otherwise-idle scalar ring) into the SAME
                # SBUF regions as w1r/x via same-tag tile rotation -- the
                # Tile WAR dep delays each load until the main loop's last
                # read of that region. All GEMM1 segments run before any
                # GEMM2 so the w2o chunks (WAR on the main loop's last x
                # read) have time to stream in.
                # w2o chunk placement: the x region holds (KH*cap)//H full
                # [P, H] chunks (different-shaped rotation of tag "x"); any
                # remainder rides in the free tail units of the w1r rotation
                # (w1o uses 4S of KF units). All three overflow-weight DMAs
                # were emitted inside the last main group's GEMM2 so their
                # sequencers reach them right as the regions' WARs clear.
                w1o_t = ovt["w1o"]

                def w2o_ap(c2, lo, hi):
                    if c2 < xch:
                        return ovt["w2o"][:, c2, lo:hi]
                    return w1o_t[:, 4 * S + (c2 - xch), lo:hi]
                ho = wts.tile([P, KF, TT], SDT, tag="h")
                offs = []
                o = 0
                for w in segs:
                    offs.append(o)
                    o += w
                for s, w in enumerate(segs):
                    for fs in range(SW // P):
                        pt = ps1.tile([P, TT], F32, tag="pt1")
                        for k in range(KH):
                            nc.tensor.matmul(
                                pt[:, :w],
                                w1o_t[:, s * 4 + fs, k * P:(k + 1) * P],
                                xo_t[:, k, offs[s]:offs[s] + w],
                                start=(k == 0),
                                stop=(k == KH - 1),
                            )
                        c = KF + s * 4 + fs
                        nc.scalar.activation(
                            ho[:, s * 4 + fs, :w], pt[:, :w], GELU,
                            bias=b1t[:, c:c + 1],
                        )
                # GEMM2 strip-major: one [P, NO] strip per output h'-chunk,
                # one store per strip (issued while later strips compute)
                for n in range(NH):
                    stage = wts.tile([P, NO], SDT, tag="ost", bufs=3)
                    for s, w in enumerate(segs):
                        # ps1 is idle during the ovf GEMM2s: alternate pools
                        # for an 8-buf rotation so copies never gate the PE
                        if (n * S + s) % 2 == 0:
                            pt2 = ps2.tile([P, TT], F32, tag="pt2")
                        else:
                            pt2 = ps1.tile([P, TT], F32, tag="pt1")
                        for k2 in range(SW // P):
                            nc.tensor.matmul(
                                pt2[:, :w],
                                w2o_ap(s * 4 + k2, n * P, (n + 1) * P),
                                ho[:, s * 4 + k2, :w],
                                start=(k2 == 0),
                                stop=(k2 == SW // P - 1),
                            )
                        # 4-matmul psum groups fill every ~200-800ns: one
                        # engine can't keep up, alternate vector/scalar drains
                        if (n * S + s) % 2 == 0:
                            nc.vector.tensor_copy(
                                stage[:, offs[s]:offs[s] + w], pt2[:, :w]
                            )
                        else:
                            nc.scalar.activation(
                                stage[:, offs[s]:offs[s] + w], pt2[:, :w],
                                mybir.ActivationFunctionType.Copy,
                            )
                    # all strip stores on sync: strips complete every ~2.5us
                    # vs ~1.2us issue cost, and sync's 16-engine ring moves a
                    # strip in ~0.6us (gpsimd's 1-engine ring took ~1.7us and
                    # dragged the kernel tail by ~10us)
                    nc.sync.dma_start(
                        ydst[:, n, cap:capx], stage[:]
                    )

    import concourse.mybir as mybir_mod

    _spill_waits(nc, mybir_mod)
    return nc


def _route(x2d, Wr, br):
    """Top-2 routing, bit-matching the reference's decisions.

    Softmax is monotonic, so top-2-of-probs == top-2-of-logits, and the
    normalized top-1 weight p0 = p1/(p1+p2) == sigmoid(l1-l2) exactly (the
    softmax denominator cancels). Ordering ties are broken by lower index,
    same as jax.lax.top_k."""
    logits = x2d @ np.asarray(Wr, np.float32) + np.asarray(br, np.float32)
    order = np.argsort(-logits, axis=-1, kind="stable")
    i1 = order[:, 0].astype(np.int64)
    i2 = order[:, 1].astype(np.int64)
    r = np.arange(logits.shape[0])
    l1 = logits[r, i1].astype(np.float64)
    l2 = logits[r, i2].astype(np.float64)
    p0 = 1.0 / (1.0 + np.exp(l2 - l1))
    return i1, i2, p0.astype(np.float32)


CM = T * 2 // E  # balanced per-core main capacity (2048, = 4*TT)


def _pack_w1(W1c):
    """[H, F] -> fs-major SBUF image [P, KF*KH*P] (fp16)."""
    return np.ascontiguousarray(
        W1c.reshape(KH, P, KF, P).transpose(1, 2, 0, 3).reshape(P, KF * KH * P)
    ).astype(np.float16)


def _pack_w2(W2c):
    """[F, H] -> n-major SBUF image [P, NH*KF*P] (fp16)."""
    return np.ascontiguousarray(
        W2c.reshape(KF, P, NH, P).transpose(1, 2, 0, 3).reshape(P, NH * KF * P)
    ).astype(np.float16)


def _prepare(x, Wr, br, W1, b1, W2, b2):
    """Route on host, build per-core input maps and the (cached) Bass program.

    Experts with more than CM tokens spill their excess into overflow
    segments, computed F-sliced across all 8 cores (see _build)."""
    x2d = np.ascontiguousarray(np.asarray(x, np.float32).reshape(T, H))
    W1 = np.asarray(W1, np.float32)
    b1 = np.asarray(b1, np.float32)
    W2 = np.asarray(W2, np.float32)

    i1, i2, p0 = _route(x2d, Wr, br)
    idxs = [np.flatnonzero((i1 == e) | (i2 == e)) for e in range(E)]
    cnts = [len(ix) for ix in idxs]
    max_cnt = max(cnts)

    def _plan(cap):
        """segments + predicted PE-span cost (ns) for a capacity choice."""
        n_seg = no = 0
        for c in cnts:
            r = max(0, c - cap)
            no += r
            n_seg += -(-r // TT)
        # feasibility: psum-bias cols, and w2o chunk homes (x region + w1r
        # tail units) for the F-sliced overflow weights
        if 4 * n_seg > KF or 4 * n_seg > (KH * cap) // H + (KF - 4 * n_seg):
            return None, None, None
        if no > 2 * TT:  # xo/stage SBUF guard
            return None, None, None
        # measured v5: 213.4ns/token main, 26.7ns/token ovf (F/8 slice),
        # ~576ns/seg instruction overhead (LDWEIGHTS-bound small matmuls)
        cost = 213.4 * cap + 26.7 * no + 576.0 * n_seg
        return cost, no, n_seg

    cands = sorted({min(CM, c) for c in cnts} | {min(CM, max_cnt)})
    best = min(
        (c for c in cands if _plan(c)[0] is not None),
        key=lambda c: _plan(c)[0],
        default=min(CM, max_cnt),
    )
    cap = best if _plan(best)[0] is not None else max_cnt

    ovf = []  # (expert, token_indices) segments of width <= TT
    for e in range(E):
        rest = idxs[e][cap:]
        while len(rest):
            ovf.append((e, rest[:TT]))
            rest = rest[TT:]
    # widest segs first: the LAST strip store is the kernel tail, and tiny
    # psum groups (drain-bound) ride behind wide ones
    ovf.sort(key=lambda et: -len(et[1]))
    segs = tuple(len(tix) for _, tix in ovf)
    if 4 * len(segs) > KF:  # extreme imbalance: fall back to plain build
        cap, ovf, segs = max_cnt, [], ()
    NO = sum(segs)

    key = (cap, segs)
    if key not in _cache:
        _cache[key] = _build(cap, segs)
    nc = _cache[key]

    xT = np.ascontiguousarray(x2d.T)  # [H, T]
    ovf_tok = (
        np.concatenate([tix for _, tix in ovf])
        if ovf else np.zeros((0,), np.int64)
    )
    x_ovf = xT[:, ovf_tok].astype(np.float16)
    SW = F // E
    in_maps = []
    for c in range(E):
        ix = idxs[c]
        xte = np.zeros((H, cap + NO), np.float16)
        n_main = min(len(ix), cap)
        xte[:, :n_main] = xT[:, ix[:n_main]]
        xte[:, cap:] = x_ovf
        b1cols = [b1[c].reshape(KF, P).T]
        for e, _ in ovf:
            b1cols.append(b1[e][c * SW:(c + 1) * SW].reshape(4, P).T)
        m = {
            "xt": xte,
            "w1": _pack_w1(W1[c]),
            "w2": _pack_w2(W2[c]),
            "b1s": np.ascontiguousarray(np.hstack(b1cols)),
        }
        if ovf:
            # per-seg fs-major images, concatenated: [P, S*4, KH*P] flat
            w1o_imgs = [
                W1[e][:, c * SW:(c + 1) * SW]
                .reshape(KH, P, 4, P).transpose(1, 2, 0, 3).reshape(P, 4 * KH * P)
                for e, _ in ovf
            ]
            m["w1o"] = np.ascontiguousarray(
                np.concatenate(w1o_imgs, axis=1)
            ).astype(np.float16)
            m["w2o"] = np.ascontiguousarray(
                np.vstack([W2[e][c * SW:(c + 1) * SW, :] for e, _ in ovf])
            ).astype(np.float16)
        in_maps.append(m)
    return nc, in_maps, (idxs, cap, ovf), p0


def _combine(res, meta, p0, b2):
    idxs, cap, ovf = meta
    b2 = np.asarray(b2, np.float32)
    out = np.zeros((T, H), np.float32)
    for e in range(E):
        ix = idxs[e][:cap]
        ye = res.results[e]["yt"][:, : len(ix)].T.astype(np.float32)  # [n_e, H]
        out[ix] += p0[ix, None] * (ye + b2[e][None, :])
    if ovf:
        y_ovf = sum(
            res.results[c]["yt"][:, cap:].astype(np.float32) for c in range(E)
        ).T  # [NO, H]
        o = 0
        for e, tix in ovf:
            w = len(tix)
            out[tix] += p0[tix, None] * (y_ovf[o:o + w] + b2[e][None, :])
            o += w
    return out.reshape(B, S, H)


def kernel(x, Wr, br, W1, b1, W2, b2):
    from concourse.bass_utils import run_bass_kernel_spmd

    nc, in_maps, meta, p0 = _prepare(x, Wr, br, W1, b1, W2, b2)
    try:
        res = run_bass_kernel_spmd(nc, in_maps, list(range(E)))
    except Exception:
        import time as _time

        _time.sleep(10)
        res = run_bass_kernel_spmd(nc, in_maps, list(range(E)))
    return _combine(res, meta, p0, b2)
